# revision 17
# baseline (speedup 1.0000x reference)
"""Distributed multi-head attention kernel for 8 TRN2 NeuronCores.

Problem: x[4, 2048, 1024] @ w_qkv[1024, 3072] -> qkv -> 16-head attention
         -> out[4, 2048, 1024], fp32.

Sharding (data parallel batch x tensor parallel heads):
  core c handles batch b = c // 2 and heads h0 = (c % 2) * 8 .. h0 + 8.
  Each core receives x_b [2048, 1024] and a host-preswizzled w block
  [ft, c, ct, f] (ft = 12 f-tiles: q pair0-3, k pair0-3, v pair0-3, so
  each f-tile is one contiguous 512KB DMA), produces U|Z tiles; the host
  does the final transpose + softmax normalization (out = U/Z).

Per-core kernel (matmuls bf16 with fp32 PSUM accumulation):
  1. x -> bf16 -> PE-transpose -> xT [c, i]
  2. qkT = w_qk.T @ x.T (per f-tile); v = x @ w_v batched per 2-pair
     group (N=256), token-major, no ones column.
  3. attention in 2-pair groups (4 heads A,B,C,D): per (ic, j):
     - dots for A,B row-packed in one [128,1024] psum (2 concurrent
       K=64 matmuls), likewise C,D in a second psum.
     - exp on ScalarE (ACT) or VectorE (bf16 Schraudolph fast-exp:
       int16(round(x*128/ln2 + bias)) bitcast bf16, ~1.8% rms) per a
       static split table, balancing both engines.
     - PV: 4 M=64 matmuls; A,B col-packed in one [128,512] psum via
       output base partitions 0/64 (concurrent col groups), likewise CD.
     - Z: 4 M=1 ones-matmuls into one psum bank at partition bases
       0/32/64/96 -> all four run concurrently on distinct col groups.
  4. evacuate U/Z psum -> SBUF bf16 -> DMA out [h, ic, 65, 512] tiles.

PSUM banks: dots 2x[128,1024] (4) + upsAB/upsCD (2) + Z (1) + shared
qkv/transpose bank (1) = 8.
"""

import math
import numpy as np

B, N, DIM = 4, 2048, 1024
HEADS, DIM_HEAD = 16, 64
INNER = HEADS * DIM_HEAD
HPC = 8                 # heads per core
FQ = HPC * DIM_HEAD     # 512 = per-core q/k/v column count
NCORES = 8

P = 128
CT = DIM // P           # 8 c-tiles (contraction)
IT = N // P             # 16 i-tiles
JT = N // P             # 16 j-tiles

# Schraudolph bf16 fast-exp: bits16 = round(dots * SCH_C1 + SCH_C2),
# bitcast int16 -> bf16 ~= exp(0.125 * dots).  SCH_C2 = 127*128 - 7.5
# (rms-optimal shift; rel err ~1.8% rms / 4.2% max).
SCH_C1 = 0.125 * 128.0 / math.log(2.0)
SCH_C2 = 127.0 * 128.0 - 7.5

_CACHE = {}


def _dve_exp(g, ic, half, j):
    """Static ACT/DVE exp split. ic0 of group 0 is PE/startup-bound with
    DVE busy on casts -> keep it all-ACT."""
    if g == 0 and ic == 0:
        return False
    if half == 0:
        return j in (3, 7, 11, 15)
    return j in (5, 13)


def _build(debug=False):
    import concourse.bass as bass
    import concourse.mybir as mybir
    import concourse.tile as tile
    from concourse import bacc
    from concourse.masks import make_identity

    f32 = mybir.dt.float32
    bf16 = mybir.dt.bfloat16
    i16 = mybir.dt.int16
    Exp = mybir.ActivationFunctionType.Exp
    Mult = mybir.AluOpType.mult
    Add = mybir.AluOpType.add
    ds = bass.ds

    nc = bacc.Bacc(None, target_bir_lowering=False)
    x_d = nc.dram_tensor("x", [N, DIM], f32, kind="ExternalInput")
    # host-preswizzled: [ft, c, ct, f]
    w_d = nc.dram_tensor("w", [12, P, CT, P], f32, kind="ExternalInput")
    o_d = nc.dram_tensor("o", [HPC, 4, 65, 512], bf16, kind="ExternalOutput")
    if debug:
        dbg_xT = nc.dram_tensor("dbg_xT", [P, CT, N], bf16, kind="ExternalOutput")
        dbg_qkT = nc.dram_tensor("dbg_qkT", [P, CT, N], bf16, kind="ExternalOutput")
        dbg_vp = nc.dram_tensor("dbg_vp", [P, JT, HPC * 64], bf16, kind="ExternalOutput")
        dbg_dt = nc.dram_tensor("dbg_dt", [P, 1024], f32, kind="ExternalOutput")
        dbg_pt = nc.dram_tensor("dbg_pt", [P, 1024], bf16, kind="ExternalOutput")

    with tile.TileContext(nc) as tc, \
         tc.tile_pool(name="persist", bufs=1) as persist, \
         tc.tile_pool(name="wload", bufs=6) as wload, \
         tc.tile_pool(name="xload", bufs=3) as xload, \
         tc.tile_pool(name="qkvp", bufs=1, space="PSUM") as qkvp, \
         tc.tile_pool(name="ptp", bufs=10) as ptp, \
         tc.tile_pool(name="uep", bufs=6) as uep:

        identf = persist.tile([P, P], f32, tag="identf", name="identf")
        make_identity(nc, identf[:])
        onesb = persist.tile([P, 1], bf16, tag="onesb", name="onesb")
        nc.vector.memset(onesb[:], 1.0)

        xT = persist.tile([P, CT, N], bf16, tag="xT", name="xT")
        wsb = persist.tile([P, CT, 3 * FQ], bf16, tag="wsb", name="wsb")
        qkT = persist.tile([P, CT, N], bf16, tag="qkT", name="qkT")
        vp = persist.tile([P, JT, HPC * 64], bf16, tag="vp", name="vp")
        vp_heads = vp[:].rearrange("p j (h c) -> p j h c", c=64)
        if debug:
            dts = persist.tile([P, 1024], f32, tag="dts", name="dts")
            pts_snap = persist.tile([P, 1024], bf16, tag="pts_snap", name="pts_snap")

        wstage = {}

        def emit_w_dma(ft):
            wfs = wload.tile([P, CT, P], f32, tag="wfs")
            nc.sync.dma_start(wfs[:], w_d[ft])
            wstage[ft] = wfs

        def emit_w_cast(ft):
            nc.vector.tensor_copy(
                wsb[:, :, ds(ft * P, P)], wstage.pop(ft)[:])

        def emit_x_tile(it, engine=None):
            xf = xload.tile([P, DIM], f32, tag="xf")
            (engine or nc.gpsimd).dma_start(xf[:], x_d[ds(it * P, P), :])
            # f32 PE transposes straight from xf (all psum stays f32 — mixing
            # bf16 transpose output into an f32 bank tears half-words); the
            # copy to xT does the bf16 cast.
            for ch in range(2):
                tpraw = qkvp.tile([P, 512], f32, tag="qkv")
                tp2 = tpraw[:].rearrange("p (k q) -> p k q", k=4)
                for k in range(4):
                    nc.tensor.transpose(
                        tp2[:, k], xf[:, ds((ch * 4 + k) * P, P)], identf[:])
                nc.vector.tensor_copy(
                    xT[:, ds(ch * 4, 4), ds(it * P, P)], tp2[:])

        def emit_qk_group(ft, ic):
            ps = qkvp.tile([P, 512], f32, tag="qkv")
            for ct in range(CT):
                nc.tensor.matmul(
                    ps[:],
                    wsb[:, ct, ds(ft * P, P)],
                    xT[:, ct, ds(ic * 512, 512)],
                    start=(ct == 0), stop=(ct == CT - 1),
                )
            nc.vector.tensor_copy(qkT[:, ft, ds(ic * 512, 512)], ps[:])

        def emit_v_group(g, it):
            # both pairs of group g: 256 v columns, token-major
            # (uniform [P,512] slot for the shared tag; use first half)
            ps = qkvp.tile([P, 512], f32, tag="qkv")
            for ct in range(CT):
                nc.tensor.matmul(
                    ps[:, 0:256],
                    xT[:, ct, ds(it * P, P)],
                    wsb[:, ct, ds(2 * FQ + g * 256, 256)],
                    start=(ct == 0), stop=(ct == CT - 1),
                )
            nc.vector.tensor_copy(
                vp_heads[:, it, ds(4 * g, 4), :],
                ps[:, 0:256].rearrange("p (h c) -> p h c", c=64),
            )

        def emit_attention_group(g, dotsp, upp, zpp, bg_emit=None):
            # pairs p0 = 2g, p1 = 2g+1; heads A..D = 4g..4g+3
            p0, p1 = 2 * g, 2 * g + 1
            for ic in range(4):
                upsAB = upp.tile([P, 512], f32, tag="upsAB")
                upsCD = upp.tile([P, 512], f32, tag="upsCD")
                zps = zpp.tile([P, 512], f32, tag="zps")
                pts = {}

                def emit_dots_exp(half, j):
                    p = p0 if half == 0 else p1
                    dt_ = dotsp.tile([P, 1024], f32, tag="dt")
                    nc.tensor.matmul(
                        dt_[:, 0:512],
                        qkT[0:64, 4 + p, ds(j * P, P)],
                        qkT[0:64, p, ds(ic * 512, 512)],
                        start=True, stop=True,
                    )
                    nc.tensor.matmul(
                        dt_[:, 512:1024],
                        qkT[64:128, 4 + p, ds(j * P, P)],
                        qkT[64:128, p, ds(ic * 512, 512)],
                        start=True, stop=True,
                    )
                    pt = ptp.tile([P, 1024], bf16, tag="pt")
                    if _dve_exp(g, ic, half, j):
                        nc.vector.tensor_scalar(
                            pt[:].bitcast(i16), dt_[:],
                            SCH_C1, SCH_C2, Mult, Add)
                    else:
                        nc.scalar.activation(pt[:], dt_[:], Exp, scale=0.125)
                    if debug and g == 0 and ic == 0 and half == 0 and j == 0:
                        nc.vector.tensor_copy(dts[:], dt_[:])
                        nc.vector.tensor_copy(pts_snap[:], pt[:])
                    pts[(half, j)] = pt

                def emit_pv_z(j):
                    ptAB = pts[(0, j)]
                    ptCD = pts[(1, j)]
                    st = dict(start=(j == 0), stop=(j == JT - 1))
                    # col-packed PV: output base partition 0/64 ->
                    # col groups {0,1} vs {2,3} -> concurrent
                    nc.tensor.matmul(
                        upsAB[0:64, :], vp_heads[:, j, 4 * g + 0, :],
                        ptAB[:, 0:512], **st)
                    nc.tensor.matmul(
                        upsAB[64:128, :], vp_heads[:, j, 4 * g + 1, :],
                        ptAB[:, 512:1024], **st)
                    nc.tensor.matmul(
                        upsCD[0:64, :], vp_heads[:, j, 4 * g + 2, :],
                        ptCD[:, 0:512], **st)
                    nc.tensor.matmul(
                        upsCD[64:128, :], vp_heads[:, j, 4 * g + 3, :],
                        ptCD[:, 512:1024], **st)
                    # Z row sums: M=1 ones-matmuls at psum partition bases
                    # 0/32/64/96 -> four distinct col groups -> concurrent
                    nc.tensor.matmul(
                        zps[0:1, :], onesb[:], ptAB[:, 0:512],
                        tile_position=(0, 0), **st)
                    nc.tensor.matmul(
                        zps[32:33, :], onesb[:], ptAB[:, 512:1024],
                        tile_position=(0, 32), **st)
                    nc.tensor.matmul(
                        zps[64:65, :], onesb[:], ptCD[:, 0:512],
                        tile_position=(0, 64), **st)
                    nc.tensor.matmul(
                        zps[96:97, :], onesb[:], ptCD[:, 512:1024],
                        tile_position=(0, 96), **st)
                    del pts[(0, j)], pts[(1, j)]

                for j2 in range(0, JT, 2):
                    if j2 == 0:
                        emit_dots_exp(0, 0)
                        if bg_emit is not None:
                            bg_emit(ic, 0)
                        emit_dots_exp(1, 0)
                        if bg_emit is not None:
                            bg_emit(ic, 1)
                        emit_dots_exp(0, 1)
                        emit_dots_exp(1, 1)
                        continue
                    if bg_emit is not None:
                        bg_emit(ic, j2)
                        bg_emit(ic, j2 + 1)
                    emit_dots_exp(0, j2)
                    emit_dots_exp(1, j2)
                    emit_pv_z(j2 - 2)
                    emit_dots_exp(0, j2 + 1)
                    emit_dots_exp(1, j2 + 1)
                    emit_pv_z(j2 - 1)
                emit_pv_z(JT - 2)
                emit_pv_z(JT - 1)

                # epilogue: evacuate U (both heads packed) + Z rows, DMA out
                ueAB = uep.tile([P, 512], bf16, tag="ue")
                nc.vector.tensor_copy(ueAB[:], upsAB[:])
                nc.gpsimd.dma_start(o_d[4 * g + 0, ic, 0:64], ueAB[0:64, :])
                nc.gpsimd.dma_start(o_d[4 * g + 1, ic, 0:64], ueAB[64:128, :])
                ueCD = uep.tile([P, 512], bf16, tag="ue")
                nc.vector.tensor_copy(ueCD[:], upsCD[:])
                nc.gpsimd.dma_start(o_d[4 * g + 2, ic, 0:64], ueCD[0:64, :])
                nc.gpsimd.dma_start(o_d[4 * g + 3, ic, 0:64], ueCD[64:128, :])
                zev = uep.tile([P, 512], bf16, tag="ue")
                nc.vector.tensor_copy(zev[:], zps[:])
                for k in range(4):
                    nc.gpsimd.dma_start(
                        o_d[4 * g + k, ic, 64:65], zev[ds(32 * k, 1), :])

        # ---- emission ----
        with tc.tile_pool(name="dotsp", bufs=2, space="PSUM") as dotsp, \
             tc.tile_pool(name="upp", bufs=1, space="PSUM") as upp, \
             tc.tile_pool(name="zpp", bufs=1, space="PSUM") as zpp:
            # w DMAs: single 512KB transfer per f-tile, priority order.
            # wload ring is 6 slots: only emit a reusing DMA after the
            # slot's previous cast has been emitted (ring deps only look
            # backward in emission order).
            for ft in (0, 4, 8, 9, 10, 11):
                emit_w_dma(ft)
            for ft in (0, 4):
                emit_w_cast(ft)
            # x tiles 0-3 split across two DMA queues for startup speed
            emit_x_tile(0, nc.gpsimd)
            emit_x_tile(1, nc.scalar)
            emit_x_tile(2, nc.gpsimd)
            emit_x_tile(3, nc.scalar)
            for ft in (8, 9, 10, 11):
                emit_w_cast(ft)
            emit_w_dma(1)
            emit_w_dma(5)
            emit_w_cast(1)
            emit_w_cast(5)
            for ft in (2, 6, 3, 7):
                emit_w_dma(ft)
            emit_qk_group(0, 0)
            emit_qk_group(4, 0)

            def bg_g0(ic, j):
                if ic != 0:
                    return
                if j == 0:
                    emit_qk_group(1, 0)
                    emit_qk_group(5, 0)
                    emit_x_tile(4)
                    emit_x_tile(5)
                elif j == 1:
                    emit_v_group(0, 0)
                    emit_v_group(0, 1)
                    emit_v_group(0, 2)
                    emit_x_tile(6)
                    emit_x_tile(7)
                elif j in (2, 3, 4, 5):
                    b = (j - 2) // 2 + 1
                    p = 0 if j % 2 == 0 else 1
                    emit_qk_group(p, b)
                    emit_qk_group(4 + p, b)
                    emit_x_tile(2 * j + 4)
                    emit_x_tile(2 * j + 5)
                    emit_v_group(0, j + 1)
                elif j in (6, 7):
                    p = 0 if j % 2 == 0 else 1
                    emit_qk_group(p, 3)
                    emit_qk_group(4 + p, 3)
                    emit_v_group(0, j + 1)
                elif j + 1 < JT:
                    emit_v_group(0, j + 1)

            def bg_g1():
                groups = [lambda: emit_w_cast(2), lambda: emit_w_cast(6),
                          lambda: emit_w_cast(3), lambda: emit_w_cast(7)]
                for icq in range(4):
                    for p in (2, 3):
                        groups.append(lambda p=p, icq=icq: emit_qk_group(p, icq))
                        groups.append(
                            lambda p=p, icq=icq: emit_qk_group(4 + p, icq))
                for itv in range(IT):
                    groups.append(lambda itv=itv: emit_v_group(1, itv))
                gi = {"i": 0}

                def bg(ic, j):
                    if ic == 0:
                        return
                    if gi["i"] < len(groups):
                        groups[gi["i"]]()
                        gi["i"] += 1
                bg.flush = lambda: [gr() for gr in groups[gi["i"]:]]
                return bg

            nxt = bg_g1()

            def bg_combined(ic, j):
                bg_g0(ic, j)
                nxt(ic, j)

            emit_attention_group(0, dotsp, upp, zpp, bg_emit=bg_combined)
            nxt.flush()
            emit_attention_group(1, dotsp, upp, zpp, bg_emit=None)
            if debug:
                nc.sync.dma_start(dbg_xT[:], xT[:])
                nc.sync.dma_start(dbg_qkT[:], qkT[:])
                nc.sync.dma_start(dbg_vp[:], vp[:])
                nc.sync.dma_start(dbg_dt[:], dts[:])
                nc.sync.dma_start(dbg_pt[:], pts_snap[:])

    nc.finalize()
    return nc


def _get_nc():
    if "nc" not in _CACHE:
        _CACHE["nc"] = _build()
    return _CACHE["nc"]


def _shard_w(w_qkv, hh):
    """Per-core w block, host-preswizzled to [ft, c, ct, f]."""
    qo = hh * FQ
    ws = np.concatenate(
        [w_qkv[:, qo:qo + FQ],
         w_qkv[:, INNER + qo:INNER + qo + FQ],
         w_qkv[:, 2 * INNER + qo:2 * INNER + qo + FQ]], axis=1)  # [1024, 1536]
    # [c=1024, f=1536] -> [ct, 128c, ft, 128f] -> [ft, c, ct, f]
    ws = ws.reshape(CT, P, 12, P).transpose(2, 1, 0, 3)
    return np.ascontiguousarray(ws)


def _unshard(results):
    """Assemble [B, N, INNER] f32 from per-core o tiles [HPC, 4, 65, 512]
    (U rows 0:64, Z row 64): host-side softmax normalize + transpose."""
    out = np.empty((B, N, INNER), np.float32)
    for c in range(NCORES):
        b, hh = c // 2, c % 2
        o = np.asarray(results[c]["o"], dtype=np.float32)  # [8, 4, 65, 512]
        u = o[:, :, 0:64, :]                   # [h, ic, d, i]
        z = o[:, :, 64:65, :]
        w = u / z                              # normalize
        # [h, ic, d, i] -> [ic*512+i, h*64+d]
        w = w.transpose(1, 3, 0, 2).reshape(N, FQ)
        out[b, :, hh * FQ:(hh + 1) * FQ] = w
    return out


def kernel(x: np.ndarray, w_qkv: np.ndarray) -> np.ndarray:
    from concourse.bass_utils import run_bass_kernel_spmd

    x = np.asarray(x, dtype=np.float32)
    w_qkv = np.asarray(w_qkv, dtype=np.float32)

    in_maps = []
    for c in range(NCORES):
        b, hh = c // 2, c % 2
        in_maps.append({
            "x": np.ascontiguousarray(x[b]),
            "w": _shard_w(w_qkv, hh),
        })

    nc = _get_nc()
    res = None
    last_err = None
    for attempt in range(3):
        try:
            res = run_bass_kernel_spmd(nc, in_maps, core_ids=list(range(NCORES)))
            break
        except Exception as e:  # transient axon/NRT device errors
            last_err = e
            import time
            time.sleep(20 * (attempt + 1))
    if res is None:
        raise last_err

    return _unshard(res.results)
